# revision 15
# baseline (speedup 1.0000x reference)
"""Trainium2 Bass kernel for nn_BoundaryAwareLoss (8 NeuronCores).

Sharding: B*H = 2*512 = 1024 rows split into 8 slabs of 128 rows; core c
handles batch b = c//4, rows [128*(c%4), 128*(c%4)+128).

Launch 1 (unchanged): sobel boundary weights + weighted cross-entropy
partials + per-k embedding segment sums (accumulating PE matmuls of
onehot[128,16]^T @ emb[128,32]).

Launch 2 (transposed-layout rewrite): instead of a 16-way broadcast
predicated-copy gather of centers per pixel (vector-bound, ~550us), the
embeddings are laid out with (row-block jb, dim d) on partitions and pixels
on the free axis:
  instT [128, F] bf16, partition p = jb*32+d, free f = (r%32)*W + w
  oh64  [64, F]  bf16, partition q = jb*16+k, host-built onehot(label==k)
  psA[128,CH] = statcg^T @ oh64-chunk      (PE gathers the center per pixel)
  diff        = instT - psA                (vector, from PSUM)
  sqd         = diff*diff                  (vector bf16 2x)
  psB[4,CH]   = jbsel^T @ sqd              (PE reduces over d -> ||e-c||^2)
d2 [4, F] is staged to DRAM, redistributed to pixel layout [128, W], then
sqrt/hinge and 16 masked scalar_tensor_tensor row-sums produce hs[128, 16];
the host sums partitions/cores and assembles the final 4 scalars.
"""

import sys

if "/opt/trn_rl_repo" not in sys.path:
    sys.path.insert(0, "/opt/trn_rl_repo")

from contextlib import ExitStack

import ml_dtypes
import numpy as np

import concourse.bass as bass
import concourse.tile as tile
from concourse import bacc, mybir
from concourse.bass_utils import run_bass_kernel_spmd

BF16 = mybir.dt.bfloat16
F32 = mybir.dt.float32

NUM_CLASSES = 19
K = 16
D = 32
B, H, W = 2, 512, 1024
ROWS = 128          # rows per core
F = 32 * W          # pixels per row-block (jb)
CH = 512
SGRP = 4
NDMA = 8
DELTA_V = 0.5
DELTA_D = 1.5

_cache = {}


def _build_launch1():
    nc = bacc.Bacc("TRN2", target_bir_lowering=False, debug=False, num_devices=8)
    sem_t = nc.dram_tensor("sem_t", [ROWS, W * NUM_CLASSES], BF16, kind="ExternalInput").ap()
    lab3 = nc.dram_tensor("lab3", [3, ROWS, W + 2], F32, kind="ExternalInput").ap()
    ilab = nc.dram_tensor("ilab", [ROWS, W], F32, kind="ExternalInput").ap()
    inst_t = nc.dram_tensor("inst_t", [ROWS, W * D], BF16, kind="ExternalInput").ap()
    o_ce = nc.dram_tensor("o_ce", [ROWS, 1], F32, kind="ExternalOutput").ap()
    o_w = nc.dram_tensor("o_w", [ROWS, 1], F32, kind="ExternalOutput").ap()
    o_sums = nc.dram_tensor("o_sums", [K, D], F32, kind="ExternalOutput").ap()

    with tile.TileContext(nc) as tc, ExitStack() as ctx:
        big = ctx.enter_context(tc.tile_pool(name="big", bufs=1))
        sml = ctx.enter_context(tc.tile_pool(name="sml", bufs=1))
        pp = ctx.enter_context(tc.tile_pool(name="pp", bufs=1, space="PSUM"))

        # small inputs first: the sync HWDGE ring is FIFO and the sobel/mask
        # vector work only needs the labels
        t_l3 = [sml.tile([ROWS, W + 2], F32, tag=f"l3_{i}", name=f"l3_{i}") for i in range(3)]
        for i in range(3):
            nc.sync.dma_start(t_l3[i][:], lab3[i])
        t_il = sml.tile([ROWS, W], F32, tag="il")
        nc.sync.dma_start(t_il[:], ilab[:])
        t_sem = big.tile([ROWS, W * NUM_CLASSES], BF16, tag="sem")
        t_inst = big.tile([ROWS, W * D], BF16, tag="inst")
        for i in range(4):
            s = slice(i * (W * NUM_CLASSES // 4), (i + 1) * (W * NUM_CLASSES // 4))
            nc.sync.dma_start(t_sem[:, s], sem_t[:, s])
            s2 = slice(i * (W * D // 4), (i + 1) * (W * D // 4))
            nc.sync.dma_start(t_inst[:, s2], inst_t[:, s2])

        # ---- sobel boundary weights (zero-padded 3x3, labels as float) ----
        lm, l0, lp = t_l3
        dh = [sml.tile([ROWS, W], F32, tag=f"dh{i}", name=f"dh{i}") for i in range(3)]
        for i, t in enumerate(t_l3):
            nc.vector.tensor_sub(dh[i][:], t[:, 2:], t[:, :-2])
        gx = sml.tile([ROWS, W], F32, tag="gx")
        nc.vector.tensor_add(gx[:], dh[0][:], dh[2][:])
        nc.vector.tensor_scalar_mul(dh[1][:], dh[1][:], 2.0)
        nc.vector.tensor_add(gx[:], gx[:], dh[1][:])
        sh = [sml.tile([ROWS, W], F32, tag=f"sh{i}", name=f"sh{i}") for i in range(2)]
        for i, t in enumerate((lm, lp)):
            nc.vector.tensor_add(sh[i][:], t[:, 2:], t[:, :-2])
            tmp = sml.tile([ROWS, W], F32, tag="shtmp")
            nc.vector.tensor_scalar_mul(tmp[:], t[:, 1:-1], 2.0)
            nc.vector.tensor_add(sh[i][:], sh[i][:], tmp[:])
        gy = sml.tile([ROWS, W], F32, tag="gy")
        nc.vector.tensor_sub(gy[:], sh[1][:], sh[0][:])
        nc.vector.tensor_mul(gx[:], gx[:], gx[:])
        nc.vector.tensor_mul(gy[:], gy[:], gy[:])
        nc.vector.tensor_add(gx[:], gx[:], gy[:])  # gx = mag^2
        wts = sml.tile([ROWS, W], F32, tag="wts")
        nc.vector.tensor_scalar(wts[:], gx[:], 0.01, None, op0=mybir.AluOpType.is_gt)
        nc.vector.tensor_scalar_add(wts[:], wts[:], 1.0)

        # ---- CE (class-major layout): gather x_label via contiguous
        # predicated copies, in-place exp, plane-wise sumexp ----
        gath = sml.tile([ROWS, W], BF16, tag="gath")
        mask = sml.tile([ROWS, W], mybir.dt.uint8, tag="mask")
        nc.vector.tensor_copy(gath[:], t_sem[:, 0:W])
        for c in range(1, NUM_CLASSES):
            nc.vector.tensor_scalar(mask[:], l0[:, 1:-1], float(c), None,
                                    op0=mybir.AluOpType.is_equal)
            nc.vector.copy_predicated(gath[:], mask[:],
                                      t_sem[:, c * W:(c + 1) * W])
        nc.scalar.activation(t_sem[:], t_sem[:], mybir.ActivationFunctionType.Exp)
        sume = sml.tile([ROWS, W], F32, tag="sume")
        nc.vector.tensor_add(sume[:], t_sem[:, 0:W], t_sem[:, W:2 * W])
        for c in range(2, NUM_CLASSES):
            nc.vector.tensor_add(sume[:], sume[:], t_sem[:, c * W:(c + 1) * W])
        logz = sml.tile([ROWS, W], F32, tag="logz")
        nc.scalar.activation(logz[:], sume[:], mybir.ActivationFunctionType.Ln)
        nll = sml.tile([ROWS, W], F32, tag="nll")
        nc.vector.tensor_sub(nll[:], logz[:], gath[:])
        nc.vector.tensor_mul(nll[:], nll[:], wts[:])
        ce_p = sml.tile([ROWS, 1], F32, tag="cep")
        nc.vector.reduce_sum(ce_p[:], nll[:], axis=mybir.AxisListType.X)
        w_p = sml.tile([ROWS, 1], F32, tag="wp")
        nc.vector.reduce_sum(w_p[:], wts[:], axis=mybir.AxisListType.X)
        nc.sync.dma_start(o_ce[:], ce_p[:])
        nc.sync.dma_start(o_w[:], w_p[:])

        # ---- instance segment sums: onehot build + accumulating PE matmuls ----
        oh = big.tile([ROWS, W * K], BF16, tag="oh")
        oh3 = oh[:].rearrange("p (w k) -> p w k", k=K)
        for k in range(K):
            nc.vector.tensor_scalar(oh3[:, :, k], t_il[:], float(k), None,
                                    op0=mybir.AluOpType.is_equal)
        ps = pp.tile([K, D], F32, tag="ps")
        inst3 = t_inst[:].rearrange("p (w d) -> p w d", d=D)
        for j in range(W):
            nc.tensor.matmul(ps[:], oh3[:, j, :], inst3[:, j, :],
                             start=(j == 0), stop=(j == W - 1))
        sums_sb = sml.tile([K, D], F32, tag="sums_sb")
        nc.vector.tensor_copy(sums_sb[:], ps[:])
        nc.sync.dma_start(o_sums[:], sums_sb[:])
    nc.compile()
    return nc


def _build_launch2():
    nc = bacc.Bacc("TRN2", target_bir_lowering=False, debug=False, num_devices=8)
    instT = nc.dram_tensor("instT", [ROWS, F], BF16, kind="ExternalInput").ap()
    oh64 = nc.dram_tensor("oh64", [64, F], BF16, kind="ExternalInput").ap()
    ilab = nc.dram_tensor("ilab2", [ROWS, W], BF16, kind="ExternalInput").ap()
    statcg = nc.dram_tensor("statcg", [64, 128], BF16, kind="ExternalInput").ap()
    jbsel = nc.dram_tensor("jbsel", [128, 4], BF16, kind="ExternalInput").ap()
    o_hs = nc.dram_tensor("o_hs", [ROWS, K], F32, kind="ExternalOutput").ap()

    AL = mybir.AluOpType
    AF = mybir.ActivationFunctionType

    with tile.TileContext(nc) as tc, ExitStack() as ctx:
        per = ctx.enter_context(tc.tile_pool(name="per", bufs=1))
        dram = ctx.enter_context(tc.tile_pool(name="dram", bufs=1, space="DRAM"))

        # small inputs first (sync HWDGE ring is FIFO; the first matmul waits
        # on statcg, so it must not queue behind the 12MB of big loads)
        t_statcg = per.tile([64, 128], BF16, tag="statcg", name="statcg_t")
        nc.sync.dma_start(t_statcg[:], statcg[:])
        t_jbsel = per.tile([128, 4], BF16, tag="jbsel", name="jbsel_t")
        nc.sync.dma_start(t_jbsel[:], jbsel[:])
        t_il = per.tile([ROWS, W], BF16, tag="il", name="il_t")
        nc.sync.dma_start(t_il[:], ilab[:])
        t_instT = per.tile([ROWS, F], BF16, tag="instT", name="instT_t")
        t_oh = per.tile([64, F], BF16, tag="oh64", name="oh64_t")
        for i in range(NDMA):
            s = slice(i * (F // NDMA), (i + 1) * (F // NDMA))
            nc.sync.dma_start(t_oh[:, s], oh64[:, s])
            nc.sync.dma_start(t_instT[:, s], instT[:, s])

        t_d2p = per.tile([ROWS, W], F32, tag="d2p", name="d2p_t")
        d2_dram = dram.tile([4, F], F32, tag="d2d", name="d2d")

        t_m16 = per.tile([ROWS, K * W], BF16, tag="m16", name="m16_t")
        for k in range(K):
            nc.vector.tensor_scalar(t_m16[:, k * W:(k + 1) * W], t_il[:],
                                    float(k), None, op0=AL.is_equal)

        # preload the Sqrt activation table off the critical path
        t_pre = per.tile([1, 1], F32, tag="pre", name="pre_t")
        nc.scalar.activation(t_pre[:], t_statcg[0:1, 0:1], AF.Sqrt)

        with tc.tile_pool(name="chk", bufs=8) as chk, \
             tc.tile_pool(name="stg", bufs=2) as stg, \
             tc.tile_pool(name="pa", bufs=5, space="PSUM") as pa, \
             tc.tile_pool(name="pb", bufs=2, space="PSUM") as pb:
            for g in range(F // (CH * SGRP)):
                stage = stg.tile([4, CH * SGRP], F32, tag="stage", name="stage")
                psAs, sqds = [], []
                for s in range(SGRP):
                    c = g * SGRP + s
                    psA = pa.tile([128, CH], F32, tag="psA", name="psA")
                    nc.tensor.matmul(psA[:], t_statcg[:],
                                     t_oh[:, c * CH:(c + 1) * CH],
                                     start=True, stop=True)
                    psAs.append(psA)
                for s in range(SGRP):
                    c = g * SGRP + s
                    sl = slice(c * CH, (c + 1) * CH)
                    diff = chk.tile([128, CH], BF16, tag="diff", name="diff")
                    nc.vector.tensor_sub(diff[:], t_instT[:, sl], psAs[s][:])
                    sqd = chk.tile([128, CH], BF16, tag="sqd", name="sqd")
                    nc.vector.tensor_mul(sqd[:], diff[:], diff[:])
                    sqds.append(sqd)
                for s in range(SGRP):
                    psB = pb.tile([4, CH], F32, tag="psB", name="psB")
                    nc.tensor.matmul(psB[:], t_jbsel[:], sqds[s][:],
                                     start=True, stop=True)
                    nc.scalar.activation(stage[:, s * CH:(s + 1) * CH], psB[:],
                                         AF.Copy)
                nc.sync.dma_start(
                    d2_dram[:, g * CH * SGRP:(g + 1) * CH * SGRP], stage[:])

        # redistribute [4, F] -> [128, W] (pixel layout) in 2 halves x 4 jb
        for h in range(2):
            for jb in range(4):
                nc.sync.dma_start(
                    t_d2p[jb * 32 + h * 16:jb * 32 + h * 16 + 16, :],
                    d2_dram[jb, h * (F // 2):(h + 1) * (F // 2)]
                    .rearrange("(r w) -> r w", w=W))

        with tc.tile_pool(name="tail", bufs=1) as tl:
            t_dist = tl.tile([ROWS, W], BF16, tag="dist", name="dist_t")
            nc.scalar.activation(t_dist[:], t_d2p[:], AF.Sqrt)
            nc.vector.tensor_scalar(t_dist[:], t_dist[:], DELTA_V, 0.0,
                                    op0=AL.subtract, op1=AL.max)
            nc.vector.tensor_mul(t_dist[:], t_dist[:], t_dist[:])
            t_hs = tl.tile([ROWS, K], F32, tag="hs", name="hs_t")
            scr = tl.tile([ROWS, W], BF16, tag="scr", name="scr_t")
            for k in range(K):
                nc.vector.scalar_tensor_tensor(
                    out=scr[:], in0=t_dist[:], scalar=1.0,
                    in1=t_m16[:, k * W:(k + 1) * W],
                    op0=AL.mult, op1=AL.mult,
                    accum_out=t_hs[:, k:k + 1])
            nc.sync.dma_start(o_hs[:], t_hs[:])
    nc.compile()
    return nc


def _get_programs():
    if "l1" not in _cache:
        _cache["l1"] = _build_launch1()
        _cache["l2"] = _build_launch2()
    return _cache["l1"], _cache["l2"]


_JBSEL = np.kron(np.eye(4, dtype=np.float32), np.ones((32, 1), np.float32))


def kernel(semantic_logits, instance_logits, semantic_labels, instance_labels,
           _return_time=False):
    nc1, nc2 = _get_programs()
    bf16 = ml_dtypes.bfloat16
    cores = list(range(8))

    lab_pad = np.zeros((B, H + 2, W + 2), np.float32)
    lab_pad[:, 1:-1, 1:-1] = semantic_labels.astype(np.float32)

    in1, in2 = [], []
    for c in cores:
        b, r0 = c // 4, 128 * (c % 4)
        # class-major: [128 rows, C*W] with each class plane contiguous
        sem = semantic_logits[b, :, r0:r0 + ROWS, :].transpose(1, 0, 2)
        inst = instance_logits[b, :, r0:r0 + ROWS, :]          # (D, 128, W)
        il = instance_labels[b, r0:r0 + ROWS, :]
        lab3 = np.stack([lab_pad[b, r0 + i:r0 + i + ROWS] for i in range(3)])
        in1.append({
            "sem_t": np.ascontiguousarray(sem).reshape(ROWS, -1).astype(bf16),
            "lab3": np.ascontiguousarray(lab3),
            "ilab": il.astype(np.float32),
            "inst_t": np.ascontiguousarray(
                inst.transpose(1, 2, 0)).reshape(ROWS, -1).astype(bf16),
        })
        # transposed layout for launch 2: partition jb*32+d, free (r%32)*W+w
        instT = inst.reshape(D, 4, 32, W).transpose(1, 0, 2, 3).reshape(ROWS, F)
        labT = il.reshape(4, F)
        oh64 = np.zeros((64, F), np.float32)
        for jb in range(4):
            for k in range(K):
                oh64[jb * 16 + k] = labT[jb] == k
        in2.append({
            "instT": instT.astype(bf16),
            "oh64": oh64.astype(bf16),
            "ilab2": il.astype(np.float32).astype(bf16),
            "jbsel": _JBSEL.astype(bf16),
        })

    r1 = run_bass_kernel_spmd(nc1, in1, core_ids=cores, trace=_return_time,
                              trace_cores=cores if _return_time else None)

    # host: combine tiny partials -> centers
    counts = np.stack([np.bincount(instance_labels[b].ravel(), minlength=K)
                       for b in range(B)]).astype(np.float32)
    sums = np.zeros((B, K, D), np.float32)
    ce_num = 0.0
    w_sum = 0.0
    for c in cores:
        sums[c // 4] += r1.results[c]["o_sums"]
        ce_num += float(r1.results[c]["o_ce"].sum())
        w_sum += float(r1.results[c]["o_w"].sum())
    centers = sums / np.maximum(counts, 1.0)[:, :, None]

    for c in cores:
        statcg = np.zeros((64, 128), np.float32)
        for jb in range(4):
            statcg[jb * 16:(jb + 1) * 16, jb * 32:(jb + 1) * 32] = centers[c // 4]
        in2[c]["statcg"] = statcg.astype(bf16)
    r2 = run_bass_kernel_spmd(nc2, in2, core_ids=cores, trace=_return_time,
                              trace_cores=cores if _return_time else None)

    hsum = np.zeros((B, K), np.float32)
    for c in cores:
        hsum[c // 4] += r2.results[c]["o_hs"].sum(axis=0)

    # final scalar assembly (identical math to the reference)
    present = (counts > 0) & (np.arange(K)[None, :] != 0)
    var_k = hsum / np.maximum(counts, 1.0) * present
    n_var = present.sum()
    loss_var = var_k.sum() / max(n_var, 1.0)
    loss_dist_n, n_dist = 0.0, 0
    for b in range(B):
        cd = centers[b][:, None, :] - centers[b][None, :, :]
        sq = (cd * cd).sum(-1)
        pair = present[b][:, None] & present[b][None, :] & ~np.eye(K, dtype=bool)
        pd = np.sqrt(np.where(pair, sq, 1.0))
        dh = np.square(np.maximum(2.0 * DELTA_D - pd, 0.0)) * pair
        n_pairs = pair.sum()
        dl = dh.sum() / max(n_pairs, 1.0)
        if present[b].sum() > 1:
            loss_dist_n += dl
            n_dist += 1
    loss_dist = loss_dist_n / max(n_dist, 1)
    instance_loss = loss_var + loss_dist
    semantic_loss = ce_num / (w_sum + 1e-8)
    mean_pw = w_sum / (B * H * W)
    total = semantic_loss + instance_loss
    out = np.array([total, semantic_loss, instance_loss, mean_pw], np.float32)
    if _return_time:
        return out, (r1.exec_time_ns, r2.exec_time_ns)
    return out
